# revision 34
# baseline (speedup 1.0000x reference)
"""Multi-head causal attention (B=2, T=4096, H=8, D=64) on 8 TRN2 NeuronCores.

Sharding: core c handles batch b = c//4 and heads (2*(c%4), 2*(c%4)+1).
Each core computes QKV for its 2 heads, causal flash-attention in an
S^T layout (keys on partitions, queries on free dim; exp on ACT; softmax
denominators via a ones-augmented V matmul), and its partial output
projection. Host sums the 4 per-batch partials and adds b_proj.

The attention inner loop is software-pipelined: S^T matmul blocks are
emitted one block ahead of their exp/mask/PV consumers so the PE stream
does not stall on ACT. PSUM budget (8 banks): S^T slots 3+2 (alternating
3-chunk/2-chunk blocks), psO accumulators 2, shared qkv/v/proj slot 1.
"""

import os
import sys

for _p in ("/opt/trn_rl_repo", "/root/.axon_site/_ro/trn_rl_repo"):
    if os.path.isdir(_p) and _p not in sys.path:
        sys.path.insert(0, _p)
        break

from contextlib import ExitStack

import ml_dtypes
import numpy as np

B, T, H, D = 2, 4096, 8, 64
C = H * D  # 512
NQT = T // 512  # 8 q-tiles of 512 queries
NKC = T // 128  # 32 k-chunks of 128 keys

# "fast": P and V in bf16 (half SBUF, same PE cost).
MODE = os.environ.get("ATTN_MODE", "fast")

_cache = {}


def _build(has_bias=True):
    import concourse.mybir as mybir
    import concourse.tile as tile
    from concourse import bacc

    f32 = mybir.dt.float32
    f32r = mybir.dt.float32r
    bf16 = mybir.dt.bfloat16
    pdt = bf16 if MODE == "fast" else f32
    Exp = mybir.ActivationFunctionType.Exp

    nc = bacc.Bacc("TRN2", target_bir_lowering=False, debug=False,
                   enable_asserts=False)

    xt_d = nc.dram_tensor("xt", [C, T], f32r, kind="ExternalInput").ap()
    wqk_d = nc.dram_tensor("wqk", [C, 256], f32r, kind="ExternalInput").ap()
    wv_d = nc.dram_tensor("wv", [C, 128], bf16, kind="ExternalInput").ap()
    bqk_d = nc.dram_tensor("bqk", [128, 2], f32, kind="ExternalInput").ap()
    bv_d = nc.dram_tensor("bv", [1, 128], bf16, kind="ExternalInput").ap()
    wp_d = nc.dram_tensor("wp", [128, C], f32r, kind="ExternalInput").ap()
    mask_d = nc.dram_tensor("mask", [128, 2048], pdt,
                            kind="ExternalInput").ap()
    out_d = nc.dram_tensor("partial", [T, C], f32, kind="ExternalOutput").ap()
    dbg = os.environ.get("ATTN_DEBUG") == "1"
    if dbg:
        dbg_qt = nc.dram_tensor("dbg_qt", [128, T], f32r,
                                kind="ExternalOutput").ap()
        dbg_kt = nc.dram_tensor("dbg_kt", [128, T], f32r,
                                kind="ExternalOutput").ap()
        dbg_va = nc.dram_tensor("dbg_va", [128, NKC * 65], pdt,
                                kind="ExternalOutput").ap()
        dbg_ot = nc.dram_tensor("dbg_ot", [128, T], f32r,
                                kind="ExternalOutput").ap()

    with tile.TileContext(nc, trace_sim=False) as tc, ExitStack() as ctx:
        cp = ctx.enter_context(tc.tile_pool(name="const", bufs=1))
        acc = ctx.enter_context(tc.tile_pool(name="acc", bufs=1,
                                             space="PSUM"))
        po_pool = ctx.enter_context(tc.tile_pool(name="po", bufs=1,
                                                 space="PSUM"))
        sp = ctx.enter_context(tc.tile_pool(name="spsum", bufs=1,
                                            space="PSUM"))
        pp = ctx.enter_context(tc.tile_pool(name="pbuf", bufs=4))
        wk = ctx.enter_context(tc.tile_pool(name="wrk", bufs=2))

        def const(shape, dt, tag):
            return cp.tile(shape, dt, tag=tag, name=tag)

        xtf = [const([128, T], f32r, f"xtf{i}") for i in range(4)]
        wqk = [const([128, 256], f32r, f"wqk{i}") for i in range(4)]
        wv = [const([128, 128], bf16, f"wv{i}") for i in range(4)]
        bqk = const([128, 2], f32, "bqk")
        bv = const([1, 128], bf16, "bv")
        wpf = const([128, C], f32r, "wpf")
        mask = const([128, 2048], pdt, "mask")
        ones1 = const([1, 128], bf16, "ones1")
        qT = const([128, T], f32r, "qT")
        kT = const([128, T], f32r, "kT")
        vaug = [const([128, NKC * 65], pdt, f"vaug{h}") for h in range(2)]
        oTS = const([128, T], f32r, "oTS")
        oT1 = const([64, T], f32r, "oT1")

        # DMA order = first-use order: per-c-chunk interleave so the
        # first qk matmul starts after ~384KB instead of ~1.5MB
        s0 = slice(0, 512)
        for i in range(4):
            nc.sync.dma_start(wqk[i][:], wqk_d[i * 128:(i + 1) * 128, :])
            nc.sync.dma_start(xtf[i][:, s0], xt_d[i * 128:(i + 1) * 128, s0])
        nc.sync.dma_start(bqk[:], bqk_d[:])
        for i in range(4):
            nc.sync.dma_start(wv[i][:], wv_d[i * 128:(i + 1) * 128, :])
        nc.sync.dma_start(bv[:], bv_d[:])
        nc.sync.dma_start(mask[:], mask_d[:])
        s1 = slice(512, 1024)
        for i in range(4):
            nc.sync.dma_start(xtf[i][:, s1], xt_d[i * 128:(i + 1) * 128, s1])
        nc.sync.dma_start(wpf[:], wp_d[:])
        nc.vector.memset(ones1[:], 1.0)
        nc.vector.memset(vaug[0][:], 1.0)
        nc.vector.memset(vaug[1][:], 1.0)

        def emit_qk(g, t):
            dst = qT if g == 0 else kT
            ps = acc.tile([128, 512], f32, tag="acc", name=f"qk{g}_{t}")
            for ci in range(4):
                nc.tensor.matmul(
                    ps[:],
                    lhsT=wqk[ci][:, g * 128:(g + 1) * 128],
                    rhs=xtf[ci][:, t * 512:(t + 1) * 512],
                    start=(ci == 0), stop=(ci == 3))
            if has_bias:
                nc.vector.tensor_scalar_add(
                    dst[:, t * 512:(t + 1) * 512], ps[:], bqk[:, g:g + 1])
            else:
                nc.vector.tensor_copy(dst[:, t * 512:(t + 1) * 512], ps[:])

        def emit_v(t, xit):
            ps = acc.tile([128, 512], f32, tag="acc", name=f"v{t}")
            psv = ps[:, 0:128]
            o = (t % 4) * 128
            for ci in range(4):
                nc.tensor.matmul(
                    psv, lhsT=xit[ci][:, o:o + 128],
                    rhs=wv[ci][:], start=(ci == 0),
                    stop=(not has_bias and ci == 3))
            if has_bias:
                nc.tensor.matmul(psv, lhsT=ones1[:], rhs=bv[:],
                                 start=False, stop=True)
            for h in range(2):
                nc.vector.tensor_copy(vaug[h][:, t * 65:t * 65 + 64],
                                      psv[:, h * 64:h * 64 + 64])

        def emit_proj(ti, alt=False):
            tagn = "po" if alt else "acc"
            pooln = po_pool if alt else acc
            psP = pooln.tile([128, 512], f32, tag=tagn, name=f"pj{ti}")
            nc.tensor.matmul(psP[:],
                             lhsT=oTS[:, ti * 128:(ti + 1) * 128],
                             rhs=wpf[:], start=True, stop=True)
            ob = wk.tile([128, 512], f32, tag="ob", bufs=5,
                         name=f"ob{ti}")
            nc.vector.tensor_copy(ob[:], psP[:])
            nc.sync.dma_start(out_d[ti * 128:(ti + 1) * 128, :], ob[:])

        # attention blocks, qi-major; alternate 3-chunk / 2-chunk PSUM slots
        class Blk:
            pass

        blocks = []
        slot = 0
        for qi in range(NQT):
            for h in range(2):
                nkc = 4 * qi + 4
                kc = 0
                while kc < nkc:
                    b = Blk()
                    b.h, b.qi, b.kc = h, qi, kc
                    b.len = min(3, nkc - kc)
                    b.slot = slot
                    b.first = kc == 0
                    b.last = kc + b.len == nkc
                    blocks.append(b)
                    slot ^= 1
                    kc += b.len

        psO = {}

        def emit_s(b):
            hb = b.h * 64
            b.psS = sp.tile([128, 1536], f32, tag=f"s{b.slot}", bufs=1,
                            name=f"s_{b.h}_{b.qi}_{b.kc}")
            for j in range(b.len):
                p = (b.kc + j) - 4 * b.qi
                # diag chunks p=1,2: only the valid q-suffix is ever read
                # downstream (exp/mask/PV all shrunk); p=3 stays full
                # (f32r below N=256 runs at 4 cyc/row - no win).
                off = 128 * p if p in (1, 2) else 0
                nc.tensor.matmul(
                    b.psS[:, j * 512 + off:(j + 1) * 512],
                    lhsT=kT[hb:hb + 64,
                            (b.kc + j) * 128:(b.kc + j + 1) * 128],
                    rhs=qT[hb:hb + 64,
                           b.qi * 512 + off:(b.qi + 1) * 512],
                    start=True, stop=True)

        def emit_f(b):
            h, qi = b.h, b.qi
            nkc = 4 * qi + 4
            va = vaug[h]
            P = pp.tile([128, b.len * 512], pdt, tag="p",
                        name=f"p_{h}_{qi}_{b.kc}")
            p0 = b.kc - 4 * qi
            off0 = 128 * p0 if p0 >= 1 else 0
            nc.scalar.activation(P[:, off0:], b.psS[:, off0:b.len * 512],
                                 Exp)
            for j in range(b.len):
                p = (b.kc + j) - 4 * qi
                if p >= 0:
                    off = 128 * p
                    nc.vector.tensor_mul(
                        P[:, j * 512 + off:(j + 1) * 512],
                        P[:, j * 512 + off:(j + 1) * 512],
                        mask[:, p * 512 + off:(p + 1) * 512])
            if b.first:
                psO[(h, qi)] = po_pool.tile([128, 512], f32, tag="po",
                                            name=f"o_{h}_{qi}")
            po = psO[(h, qi)]
            for j in range(b.len):
                p = (b.kc + j) - 4 * qi
                off = 128 * p if p >= 1 else 0
                nc.tensor.matmul(
                    po[0:65, off:],
                    lhsT=va[:, (b.kc + j) * 65:(b.kc + j) * 65 + 65],
                    rhs=P[:, j * 512 + off:(j + 1) * 512],
                    start=(b.kc + j == 0), stop=(b.kc + j == nkc - 1))
            if b.last:
                # stage psO to SBUF fast so the PSUM slot frees early
                oU = wk.tile([65, 512], f32, tag="oU", bufs=3,
                             name=f"oU{h}_{qi}")
                nc.vector.tensor_copy(oU[:], po[0:65, :])
                rec0 = wk.tile([1, 512], f32, tag="rec0", bufs=3,
                               name=f"rc0{h}_{qi}")
                nc.sync.dma_start(rec0[:], oU[64:65, :])
                rec = wk.tile([1, 512], f32, tag="rec", bufs=3,
                               name=f"rc{h}_{qi}")
                nc.vector.reciprocal_approx_fast(rec[:], rec0[:])
                rb = wk.tile([64, 512], f32, tag="rb", bufs=3,
                              name=f"rb{h}_{qi}")
                nc.gpsimd.partition_broadcast(rb[:], rec[:])
                dstq = slice(qi * 512, (qi + 1) * 512)
                if h == 0:
                    nc.vector.tensor_mul(oTS[0:64, dstq], oU[0:64, :],
                                         rb[:])
                else:
                    nc.vector.tensor_mul(oT1[:, dstq], oU[0:64, :], rb[:])
                    nc.sync.dma_start(oTS[64:128, dstq], oT1[:, dstq])
                    for ti in range(4 * qi, 4 * qi + 4):
                        alt = qi == NQT - 1 and ti % 2 == 1
                        pending.append(
                            lambda ti=ti, alt=alt: emit_proj(ti, alt))

        # main pipelined emission: S of block i+1 before finish of block i;
        # qk/v/proj groups spread between attention blocks via pending queue
        pending = []

        def emit_inputs(t):
            if t >= NQT:
                return
            if t + 1 < NQT:
                sn = slice((t + 1) * 512, (t + 2) * 512)
                for i in range(4):
                    nc.sync.dma_start(xtf[i][:, sn],
                                      xt_d[i * 128:(i + 1) * 128, sn])
            emit_qk(0, t)
            emit_qk(1, t)
            s = slice(t * 512, (t + 1) * 512)
            xit = [wk.tile([128, 512], bf16, tag=f"xit{i}", bufs=2,
                           name=f"xit{i}_{t}") for i in range(4)]
            for i in range(4):
                nc.vector.tensor_copy(xit[i][:], xtf[i][:, s].bitcast(f32))
            for tv in range(4 * t, 4 * t + 4):
                pending.append(lambda tv=tv, xit=xit: emit_v(tv, xit))

        emit_inputs(0)
        for tv in range(0, 4):
            pending.pop(0)()
        bi = 0
        from collections import deque
        prevq = deque()
        LOOKAHEAD = 2
        for t in range(NQT):
            if pending:
                for fn in pending:
                    fn()
                pending.clear()
            emit_inputs(t + 1)
            while bi < len(blocks) and blocks[bi].qi == t:
                b = blocks[bi]
                emit_s(b)
                prevq.append(b)
                if len(prevq) > LOOKAHEAD:
                    emit_f(prevq.popleft())
                bi += 1
                if pending:
                    pending.pop(0)()
        while prevq:
            emit_f(prevq.popleft())
        for fn in pending:
            fn()
        pending.clear()

        if dbg:
            nc.sync.dma_start(dbg_qt[:], qT[:])
            nc.sync.dma_start(dbg_kt[:], kT[:])
            nc.sync.dma_start(dbg_va[:], vaug[1][:])
            nc.sync.dma_start(dbg_ot[:], oTS[:])

    nc.compile()
    return nc


def _get_nc(has_bias=True):
    key = f"nc{has_bias}"
    if key not in _cache:
        _cache[key] = _build(has_bias)
    return _cache[key]


def _prep_inputs(x, w_qkv, b_qkv, w_proj):
    x = np.asarray(x, np.float32)
    w_qkv = np.asarray(w_qkv, np.float32)
    b_qkv = np.asarray(b_qkv, np.float32)
    bf = ml_dtypes.bfloat16
    pdt_np = bf if MODE == "fast" else np.float32

    # causal mask tile: mask[k, p*512 + q] = (128*p + k <= q)
    k_idx = np.arange(128)[:, None]
    q_idx = np.arange(512)[None, :]
    mask = np.concatenate(
        [(128 * p + k_idx <= q_idx) for p in range(4)], axis=1)
    mask = mask.astype(pdt_np)

    in_maps = []
    for c in range(8):
        b = c // 4
        h0 = 2 * (c % 4)
        cols = slice(h0 * 64, (h0 + 2) * 64)  # 128 contiguous dims (2 heads)
        xt = np.ascontiguousarray(x[b].T)
        wq = w_qkv[:, :C][:, cols] * 0.125
        wkk = w_qkv[:, C:2 * C][:, cols]
        wvv = w_qkv[:, 2 * C:][:, cols]
        bq = b_qkv[:C][cols] * 0.125
        bk = b_qkv[C:2 * C][cols]
        bvv = b_qkv[2 * C:][cols]
        in_maps.append({
            "xt": xt,
            "wqk": np.ascontiguousarray(np.concatenate([wq, wkk], axis=1)),
            "wv": np.ascontiguousarray(wvv.astype(bf)),
            "bqk": np.ascontiguousarray(np.stack([bq, bk], axis=1)),
            "bv": np.ascontiguousarray(bvv[None, :].astype(bf)),
            "wp": np.ascontiguousarray(
                np.asarray(w_proj, np.float32)[cols, :]),
            "mask": mask,
        })
    return in_maps


def kernel(x, w_qkv, b_qkv, w_proj, b_proj, _want_trace=False):
    from concourse.bass_utils import run_bass_kernel_spmd

    has_bias = bool(np.any(np.asarray(b_qkv)))
    nc = _get_nc(has_bias)
    in_maps = _prep_inputs(x, w_qkv, b_qkv, w_proj)
    res = run_bass_kernel_spmd(nc, in_maps, list(range(8)),
                               trace=_want_trace)
    if _want_trace:
        _cache["last_result"] = res
    out = np.zeros((B, T, C), np.float32)
    for c in range(8):
        out[c // 4] += res.results[c]["partial"]
    out += np.asarray(b_proj, np.float32)[None, None, :]
    return out


# revision 39
# speedup vs baseline: 1.0098x; 1.0098x over previous
"""Multi-head causal attention (B=2, T=4096, H=8, D=64) on 8 TRN2 NeuronCores.

Sharding: core c handles batch b = c//4 and heads (2*(c%4), 2*(c%4)+1).
Each core computes QKV for its 2 heads, causal flash-attention in an
S^T layout (keys on partitions, queries on free dim; exp on ACT; softmax
denominators via a ones-augmented V matmul), and its partial output
projection. Host sums the 4 per-batch partials and adds b_proj.

The attention inner loop is software-pipelined: S^T matmul blocks are
emitted one block ahead of their exp/mask/PV consumers so the PE stream
does not stall on ACT. PSUM budget (8 banks): S^T slots 3+2 (alternating
3-chunk/2-chunk blocks), psO accumulators 2, shared qkv/v/proj slot 1.
"""

import os
import sys

for _p in ("/opt/trn_rl_repo", "/root/.axon_site/_ro/trn_rl_repo"):
    if os.path.isdir(_p) and _p not in sys.path:
        sys.path.insert(0, _p)
        break

from contextlib import ExitStack

import ml_dtypes
import numpy as np

B, T, H, D = 2, 4096, 8, 64
C = H * D  # 512
NQT = T // 512  # 8 q-tiles of 512 queries
NKC = T // 128  # 32 k-chunks of 128 keys

# "fast": P and V in bf16 (half SBUF, same PE cost).
MODE = os.environ.get("ATTN_MODE", "fast")

_cache = {}


def _build(has_bias=True):
    import concourse.mybir as mybir
    import concourse.tile as tile
    from concourse import bacc

    f32 = mybir.dt.float32
    f32r = mybir.dt.float32r
    bf16 = mybir.dt.bfloat16
    pdt = bf16 if MODE == "fast" else f32
    Exp = mybir.ActivationFunctionType.Exp

    nc = bacc.Bacc("TRN2", target_bir_lowering=False, debug=False,
                   enable_asserts=False)

    xt_d = nc.dram_tensor("xt", [C, T], f32r, kind="ExternalInput").ap()
    wqk_d = nc.dram_tensor("wqk", [C, 256], f32r, kind="ExternalInput").ap()
    wv_d = nc.dram_tensor("wv", [C, 128], bf16, kind="ExternalInput").ap()
    bqk_d = nc.dram_tensor("bqk", [128, 2], f32, kind="ExternalInput").ap()
    bv_d = nc.dram_tensor("bv", [1, 128], bf16, kind="ExternalInput").ap()
    wp_d = nc.dram_tensor("wp", [128, C], f32r, kind="ExternalInput").ap()
    mask_d = nc.dram_tensor("mask", [128, 2048], pdt,
                            kind="ExternalInput").ap()
    out_d = nc.dram_tensor("partial", [T, C], f32, kind="ExternalOutput").ap()
    dbg = os.environ.get("ATTN_DEBUG") == "1"
    if dbg:
        dbg_qt = nc.dram_tensor("dbg_qt", [128, T], f32r,
                                kind="ExternalOutput").ap()
        dbg_kt = nc.dram_tensor("dbg_kt", [128, T], f32r,
                                kind="ExternalOutput").ap()
        dbg_va = nc.dram_tensor("dbg_va", [128, NKC * 65], pdt,
                                kind="ExternalOutput").ap()
        dbg_ot = nc.dram_tensor("dbg_ot", [128, T], f32r,
                                kind="ExternalOutput").ap()

    with tile.TileContext(nc, trace_sim=False) as tc, ExitStack() as ctx:
        cp = ctx.enter_context(tc.tile_pool(name="const", bufs=1))
        acc = ctx.enter_context(tc.tile_pool(name="acc", bufs=1,
                                             space="PSUM"))
        po_pool = ctx.enter_context(tc.tile_pool(name="po", bufs=1,
                                                 space="PSUM"))
        sp = ctx.enter_context(tc.tile_pool(name="spsum", bufs=1,
                                            space="PSUM"))
        pp = ctx.enter_context(tc.tile_pool(name="pbuf", bufs=5))
        wk = ctx.enter_context(tc.tile_pool(name="wrk", bufs=2))

        def const(shape, dt, tag):
            return cp.tile(shape, dt, tag=tag, name=tag)

        xtf = [const([128, T], f32r, f"xtf{i}") for i in range(4)]
        wqk = [const([128, 256], f32r, f"wqk{i}") for i in range(4)]
        wv = [const([128, 128], bf16, f"wv{i}") for i in range(4)]
        bqk = const([128, 2], f32, "bqk")
        bv = const([1, 128], bf16, "bv")
        wpf = const([128, C], f32r, "wpf")
        mask = const([128, 2048], pdt, "mask")
        ones1 = const([1, 128], bf16, "ones1")
        qT = const([128, T], f32r, "qT")
        kT = const([128, T], f32r, "kT")
        vaug = [const([128, NKC * 65], pdt, f"vaug{h}") for h in range(2)]
        oTS = const([128, T], f32r, "oTS")
        oT1 = const([64, T], f32r, "oT1")

        # DMA order = first-use order: per-c-chunk interleave so the
        # first qk matmul starts after ~384KB instead of ~1.5MB
        s0 = slice(0, 512)
        for i in range(4):
            nc.sync.dma_start(wqk[i][:], wqk_d[i * 128:(i + 1) * 128, :])
            nc.sync.dma_start(xtf[i][:, s0], xt_d[i * 128:(i + 1) * 128, s0])
        nc.sync.dma_start(bqk[:], bqk_d[:])
        for i in range(4):
            nc.sync.dma_start(wv[i][:], wv_d[i * 128:(i + 1) * 128, :])
        nc.sync.dma_start(bv[:], bv_d[:])
        nc.sync.dma_start(mask[:], mask_d[:])
        s1 = slice(512, 1024)
        for i in range(4):
            nc.sync.dma_start(xtf[i][:, s1], xt_d[i * 128:(i + 1) * 128, s1])
        nc.sync.dma_start(wpf[:], wp_d[:])
        nc.vector.memset(ones1[:], 1.0)
        nc.vector.memset(vaug[0][:], 1.0)
        nc.vector.memset(vaug[1][:], 1.0)

        def emit_qk(g, t):
            dst = qT if g == 0 else kT
            ps = acc.tile([128, 512], f32, tag="acc", name=f"qk{g}_{t}")
            for ci in range(4):
                nc.tensor.matmul(
                    ps[:],
                    lhsT=wqk[ci][:, g * 128:(g + 1) * 128],
                    rhs=xtf[ci][:, t * 512:(t + 1) * 512],
                    start=(ci == 0), stop=(ci == 3))
            if has_bias:
                nc.vector.tensor_scalar_add(
                    dst[:, t * 512:(t + 1) * 512], ps[:], bqk[:, g:g + 1])
            else:
                nc.vector.tensor_copy(dst[:, t * 512:(t + 1) * 512], ps[:])

        def emit_v(t, xit):
            ps = acc.tile([128, 512], f32, tag="acc", name=f"v{t}")
            psv = ps[:, 0:128]
            o = (t % 4) * 128
            for ci in range(4):
                nc.tensor.matmul(
                    psv, lhsT=xit[ci][:, o:o + 128],
                    rhs=wv[ci][:], start=(ci == 0),
                    stop=(not has_bias and ci == 3))
            if has_bias:
                nc.tensor.matmul(psv, lhsT=ones1[:], rhs=bv[:],
                                 start=False, stop=True)
            for h in range(2):
                nc.vector.tensor_copy(vaug[h][:, t * 65:t * 65 + 64],
                                      psv[:, h * 64:h * 64 + 64])

        def emit_proj(ti, alt=False):
            tagn = "po" if alt else "acc"
            pooln = po_pool if alt else acc
            psP = pooln.tile([128, 512], f32, tag=tagn, name=f"pj{ti}")
            nc.tensor.matmul(psP[:],
                             lhsT=oTS[:, ti * 128:(ti + 1) * 128],
                             rhs=wpf[:], start=True, stop=True)
            ob = wk.tile([128, 512], f32, tag="ob", bufs=5,
                         name=f"ob{ti}")
            nc.vector.tensor_copy(ob[:], psP[:])
            nc.sync.dma_start(out_d[ti * 128:(ti + 1) * 128, :], ob[:])

        # attention blocks, qi-major; alternate 3-chunk / 2-chunk PSUM slots
        class Blk:
            pass

        blocks = []
        slot = 0
        for qi in range(NQT):
            for h in range(2):
                nkc = 4 * qi + 4
                kc = 0
                while kc < nkc:
                    b = Blk()
                    b.h, b.qi, b.kc = h, qi, kc
                    b.len = min(3, nkc - kc)
                    b.slot = slot
                    b.first = kc == 0
                    b.last = kc + b.len == nkc
                    blocks.append(b)
                    slot ^= 1
                    kc += b.len

        psO = {}

        def emit_s(b):
            hb = b.h * 64
            b.psS = sp.tile([128, 1536], f32, tag=f"s{b.slot}", bufs=1,
                            name=f"s_{b.h}_{b.qi}_{b.kc}")
            for j in range(b.len):
                p = (b.kc + j) - 4 * b.qi
                # diag chunks p=1,2: only the valid q-suffix is ever read
                # downstream (exp/mask/PV all shrunk); p=3 stays full
                # (f32r below N=256 runs at 4 cyc/row - no win).
                off = 128 * p if p in (1, 2) else 0
                nc.tensor.matmul(
                    b.psS[:, j * 512 + off:(j + 1) * 512],
                    lhsT=kT[hb:hb + 64,
                            (b.kc + j) * 128:(b.kc + j + 1) * 128],
                    rhs=qT[hb:hb + 64,
                           b.qi * 512 + off:(b.qi + 1) * 512],
                    start=True, stop=True)

        def emit_f(b):
            h, qi = b.h, b.qi
            nkc = 4 * qi + 4
            va = vaug[h]
            P = pp.tile([128, b.len * 512], pdt, tag="p",
                        name=f"p_{h}_{qi}_{b.kc}")
            p0 = b.kc - 4 * qi
            off0 = 128 * p0 if p0 >= 1 else 0
            nc.scalar.activation(P[:, off0:], b.psS[:, off0:b.len * 512],
                                 Exp)
            for j in range(b.len):
                p = (b.kc + j) - 4 * qi
                if p >= 0:
                    off = 128 * p
                    nc.vector.tensor_mul(
                        P[:, j * 512 + off:(j + 1) * 512],
                        P[:, j * 512 + off:(j + 1) * 512],
                        mask[:, p * 512 + off:(p + 1) * 512])
            if b.first:
                psO[(h, qi)] = po_pool.tile([128, 512], f32, tag="po",
                                            name=f"o_{h}_{qi}")
            po = psO[(h, qi)]
            for j in range(b.len):
                p = (b.kc + j) - 4 * qi
                off = 128 * p if p >= 1 else 0
                nc.tensor.matmul(
                    po[0:65, off:],
                    lhsT=va[:, (b.kc + j) * 65:(b.kc + j) * 65 + 65],
                    rhs=P[:, j * 512 + off:(j + 1) * 512],
                    start=(b.kc + j == 0), stop=(b.kc + j == nkc - 1))
            if b.last:
                # stage psO to SBUF fast so the PSUM slot frees early
                oU = wk.tile([65, 512], f32, tag="oU", bufs=3,
                             name=f"oU{h}_{qi}")
                nc.vector.tensor_copy(oU[:], po[0:65, :])
                rec0 = wk.tile([1, 512], f32, tag="rec0", bufs=3,
                               name=f"rc0{h}_{qi}")
                nc.sync.dma_start(rec0[:], oU[64:65, :])
                rec = wk.tile([1, 512], f32, tag="rec", bufs=3,
                               name=f"rc{h}_{qi}")
                nc.vector.reciprocal_approx_fast(rec[:], rec0[:])
                rb = wk.tile([64, 512], f32, tag="rb", bufs=3,
                              name=f"rb{h}_{qi}")
                nc.gpsimd.partition_broadcast(rb[:], rec[:])
                dstq = slice(qi * 512, (qi + 1) * 512)
                if h == 0:
                    nc.vector.tensor_mul(oTS[0:64, dstq], oU[0:64, :],
                                         rb[:])
                else:
                    nc.vector.tensor_mul(oT1[:, dstq], oU[0:64, :], rb[:])
                    nc.sync.dma_start(oTS[64:128, dstq], oT1[:, dstq])
                    for ti in range(4 * qi, 4 * qi + 4):
                        alt = qi == NQT - 1 and ti % 2 == 1
                        pending.append(
                            lambda ti=ti, alt=alt: emit_proj(ti, alt))

        # main pipelined emission: S of block i+1 before finish of block i;
        # qk/v/proj groups spread between attention blocks via pending queue
        pending = []

        def emit_inputs(t):
            if t >= NQT:
                return
            if t + 1 < NQT:
                sn = slice((t + 1) * 512, (t + 2) * 512)
                for i in range(4):
                    nc.sync.dma_start(xtf[i][:, sn],
                                      xt_d[i * 128:(i + 1) * 128, sn])
            emit_qk(0, t)
            emit_qk(1, t)
            s = slice(t * 512, (t + 1) * 512)
            xit = [wk.tile([128, 512], bf16, tag=f"xit{i}", bufs=2,
                           name=f"xit{i}_{t}") for i in range(4)]
            for i in range(4):
                nc.vector.tensor_copy(xit[i][:], xtf[i][:, s].bitcast(f32))
            for tv in range(4 * t, 4 * t + 4):
                pending.append(lambda tv=tv, xit=xit: emit_v(tv, xit))

        emit_inputs(0)
        for tv in range(0, 4):
            pending.pop(0)()
        bi = 0
        from collections import deque
        prevq = deque()
        LOOKAHEAD = 3
        for t in range(NQT):
            if pending:
                for fn in pending:
                    fn()
                pending.clear()
            emit_inputs(t + 1)
            while bi < len(blocks) and blocks[bi].qi == t:
                b = blocks[bi]
                emit_s(b)
                prevq.append(b)
                if len(prevq) > LOOKAHEAD:
                    emit_f(prevq.popleft())
                bi += 1
                if pending:
                    pending.pop(0)()
        while prevq:
            emit_f(prevq.popleft())
        for fn in pending:
            fn()
        pending.clear()

        if dbg:
            nc.sync.dma_start(dbg_qt[:], qT[:])
            nc.sync.dma_start(dbg_kt[:], kT[:])
            nc.sync.dma_start(dbg_va[:], vaug[1][:])
            nc.sync.dma_start(dbg_ot[:], oTS[:])

    nc.compile()
    return nc


def _get_nc(has_bias=True):
    key = f"nc{has_bias}"
    if key not in _cache:
        _cache[key] = _build(has_bias)
    return _cache[key]


def _prep_inputs(x, w_qkv, b_qkv, w_proj):
    x = np.asarray(x, np.float32)
    w_qkv = np.asarray(w_qkv, np.float32)
    b_qkv = np.asarray(b_qkv, np.float32)
    bf = ml_dtypes.bfloat16
    pdt_np = bf if MODE == "fast" else np.float32

    # causal mask tile: mask[k, p*512 + q] = (128*p + k <= q)
    k_idx = np.arange(128)[:, None]
    q_idx = np.arange(512)[None, :]
    mask = np.concatenate(
        [(128 * p + k_idx <= q_idx) for p in range(4)], axis=1)
    mask = mask.astype(pdt_np)

    in_maps = []
    for c in range(8):
        b = c // 4
        h0 = 2 * (c % 4)
        cols = slice(h0 * 64, (h0 + 2) * 64)  # 128 contiguous dims (2 heads)
        xt = np.ascontiguousarray(x[b].T)
        wq = w_qkv[:, :C][:, cols] * 0.125
        wkk = w_qkv[:, C:2 * C][:, cols]
        wvv = w_qkv[:, 2 * C:][:, cols]
        bq = b_qkv[:C][cols] * 0.125
        bk = b_qkv[C:2 * C][cols]
        bvv = b_qkv[2 * C:][cols]
        in_maps.append({
            "xt": xt,
            "wqk": np.ascontiguousarray(np.concatenate([wq, wkk], axis=1)),
            "wv": np.ascontiguousarray(wvv.astype(bf)),
            "bqk": np.ascontiguousarray(np.stack([bq, bk], axis=1)),
            "bv": np.ascontiguousarray(bvv[None, :].astype(bf)),
            "wp": np.ascontiguousarray(
                np.asarray(w_proj, np.float32)[cols, :]),
            "mask": mask,
        })
    return in_maps


def kernel(x, w_qkv, b_qkv, w_proj, b_proj, _want_trace=False):
    from concourse.bass_utils import run_bass_kernel_spmd

    has_bias = bool(np.any(np.asarray(b_qkv)))
    nc = _get_nc(has_bias)
    in_maps = _prep_inputs(x, w_qkv, b_qkv, w_proj)
    res = run_bass_kernel_spmd(nc, in_maps, list(range(8)),
                               trace=_want_trace)
    if _want_trace:
        _cache["last_result"] = res
    out = np.zeros((B, T, C), np.float32)
    for c in range(8):
        out[c // 4] += res.results[c]["partial"]
    out += np.asarray(b_proj, np.float32)[None, None, :]
    return out


# revision 41
# speedup vs baseline: 1.0189x; 1.0090x over previous
"""Multi-head causal attention (B=2, T=4096, H=8, D=64) on 8 TRN2 NeuronCores.

Sharding: core c handles batch b = c//4 and heads (2*(c%4), 2*(c%4)+1).
Each core computes QKV for its 2 heads, causal flash-attention in an
S^T layout (keys on partitions, queries on free dim; exp on ACT; softmax
denominators via a ones-augmented V matmul), and its partial output
projection. Host sums the 4 per-batch partials and adds b_proj.

The attention inner loop is software-pipelined: S^T matmul blocks are
emitted one block ahead of their exp/mask/PV consumers so the PE stream
does not stall on ACT. PSUM budget (8 banks): S^T slots 3+2 (alternating
3-chunk/2-chunk blocks), psO accumulators 2, shared qkv/v/proj slot 1.
"""

import os
import sys

for _p in ("/opt/trn_rl_repo", "/root/.axon_site/_ro/trn_rl_repo"):
    if os.path.isdir(_p) and _p not in sys.path:
        sys.path.insert(0, _p)
        break

from contextlib import ExitStack

import ml_dtypes
import numpy as np

B, T, H, D = 2, 4096, 8, 64
C = H * D  # 512
NQT = T // 512  # 8 q-tiles of 512 queries
NKC = T // 128  # 32 k-chunks of 128 keys

# "fast": P and V in bf16 (half SBUF, same PE cost).
MODE = os.environ.get("ATTN_MODE", "fast")

_cache = {}


def _build(has_bias=True):
    import concourse.mybir as mybir
    import concourse.tile as tile
    from concourse import bacc

    f32 = mybir.dt.float32
    f32r = mybir.dt.float32r
    bf16 = mybir.dt.bfloat16
    pdt = bf16 if MODE == "fast" else f32
    Exp = mybir.ActivationFunctionType.Exp

    nc = bacc.Bacc("TRN2", target_bir_lowering=False, debug=False,
                   enable_asserts=False)

    xt_d = nc.dram_tensor("xt", [C, T], f32r, kind="ExternalInput").ap()
    wqk_d = nc.dram_tensor("wqk", [C, 256], f32r, kind="ExternalInput").ap()
    wv_d = nc.dram_tensor("wv", [C, 128], bf16, kind="ExternalInput").ap()
    bqk_d = nc.dram_tensor("bqk", [128, 2], f32, kind="ExternalInput").ap()
    bv_d = nc.dram_tensor("bv", [1, 128], bf16, kind="ExternalInput").ap()
    wp_d = nc.dram_tensor("wp", [128, C], f32r, kind="ExternalInput").ap()
    mask_d = nc.dram_tensor("mask", [128, 2048], pdt,
                            kind="ExternalInput").ap()
    out_d = nc.dram_tensor("partial", [T, C], f32, kind="ExternalOutput").ap()
    dbg = os.environ.get("ATTN_DEBUG") == "1"
    if dbg:
        dbg_qt = nc.dram_tensor("dbg_qt", [128, T], f32r,
                                kind="ExternalOutput").ap()
        dbg_kt = nc.dram_tensor("dbg_kt", [128, T], f32r,
                                kind="ExternalOutput").ap()
        dbg_va = nc.dram_tensor("dbg_va", [128, NKC * 65], pdt,
                                kind="ExternalOutput").ap()
        dbg_ot = nc.dram_tensor("dbg_ot", [128, T], f32r,
                                kind="ExternalOutput").ap()

    with tile.TileContext(nc, trace_sim=False) as tc, ExitStack() as ctx:
        cp = ctx.enter_context(tc.tile_pool(name="const", bufs=1))
        acc = ctx.enter_context(tc.tile_pool(name="acc", bufs=1,
                                             space="PSUM"))
        po_pool = ctx.enter_context(tc.tile_pool(name="po", bufs=1,
                                                 space="PSUM"))
        sp = ctx.enter_context(tc.tile_pool(name="spsum", bufs=1,
                                            space="PSUM"))
        pp = ctx.enter_context(tc.tile_pool(name="pbuf", bufs=5))
        wk = ctx.enter_context(tc.tile_pool(name="wrk", bufs=2))

        def const(shape, dt, tag):
            return cp.tile(shape, dt, tag=tag, name=tag)

        xtf = [const([128, T], f32r, f"xtf{i}") for i in range(4)]
        wqk = [const([128, 256], f32r, f"wqk{i}") for i in range(4)]
        wv = [const([128, 128], bf16, f"wv{i}") for i in range(4)]
        bqk = const([128, 2], f32, "bqk")
        bv = const([1, 128], bf16, "bv")
        wpf = const([128, C], f32r, "wpf")
        wp1 = const([64, C], f32r, "wp1")
        mask = const([128, 2048], pdt, "mask")
        ones1 = const([1, 128], bf16, "ones1")
        qT = const([128, T], f32r, "qT")
        kT = const([128, T], f32r, "kT")
        vaug = [const([128, NKC * 65], pdt, f"vaug{h}") for h in range(2)]
        oTS = const([128, T], f32r, "oTS")
        oT1 = const([64, T], f32r, "oT1")

        # DMA order = first-use order: per-c-chunk interleave so the
        # first qk matmul starts after ~384KB instead of ~1.5MB
        s0 = slice(0, 512)
        for i in range(4):
            nc.sync.dma_start(wqk[i][:], wqk_d[i * 128:(i + 1) * 128, :])
            nc.sync.dma_start(xtf[i][:, s0], xt_d[i * 128:(i + 1) * 128, s0])
        nc.sync.dma_start(bqk[:], bqk_d[:])
        for i in range(4):
            nc.sync.dma_start(wv[i][:], wv_d[i * 128:(i + 1) * 128, :])
        nc.sync.dma_start(bv[:], bv_d[:])
        nc.sync.dma_start(mask[:], mask_d[:])
        s1 = slice(512, 1024)
        for i in range(4):
            nc.sync.dma_start(xtf[i][:, s1], xt_d[i * 128:(i + 1) * 128, s1])
        nc.sync.dma_start(wpf[:], wp_d[:])
        nc.sync.dma_start(wp1[:], wp_d[64:128, :])
        nc.vector.memset(ones1[:], 1.0)
        nc.vector.memset(vaug[0][:], 1.0)
        nc.vector.memset(vaug[1][:], 1.0)

        def emit_qk(g, t):
            dst = qT if g == 0 else kT
            ps = acc.tile([128, 512], f32, tag="acc", name=f"qk{g}_{t}")
            for ci in range(4):
                nc.tensor.matmul(
                    ps[:],
                    lhsT=wqk[ci][:, g * 128:(g + 1) * 128],
                    rhs=xtf[ci][:, t * 512:(t + 1) * 512],
                    start=(ci == 0), stop=(ci == 3))
            if has_bias:
                nc.vector.tensor_scalar_add(
                    dst[:, t * 512:(t + 1) * 512], ps[:], bqk[:, g:g + 1])
            else:
                nc.vector.tensor_copy(dst[:, t * 512:(t + 1) * 512], ps[:])

        def emit_v(t, xit):
            ps = acc.tile([128, 512], f32, tag="acc", name=f"v{t}")
            psv = ps[:, 0:128]
            o = (t % 4) * 128
            for ci in range(4):
                nc.tensor.matmul(
                    psv, lhsT=xit[ci][:, o:o + 128],
                    rhs=wv[ci][:], start=(ci == 0),
                    stop=(not has_bias and ci == 3))
            if has_bias:
                nc.tensor.matmul(psv, lhsT=ones1[:], rhs=bv[:],
                                 start=False, stop=True)
            for h in range(2):
                nc.vector.tensor_copy(vaug[h][:, t * 65:t * 65 + 64],
                                      psv[:, h * 64:h * 64 + 64])

        def emit_proj(ti, alt=False, split=False):
            tagn = "po" if alt else "acc"
            pooln = po_pool if alt else acc
            psP = pooln.tile([128, 512], f32, tag=tagn, name=f"pj{ti}")
            if split:
                # final row: read heads separately so the projs don't wait
                # on the oTS stacking DMA (PE is idle in the tail anyway)
                tc0 = slice(ti * 128, (ti + 1) * 128)
                nc.tensor.matmul(psP[:], lhsT=oTS[0:64, tc0],
                                 rhs=wpf[0:64, :], start=True, stop=False)
                nc.tensor.matmul(psP[:], lhsT=oT1[:, tc0],
                                 rhs=wp1[:], start=False, stop=True)
            else:
                nc.tensor.matmul(psP[:],
                                 lhsT=oTS[:, ti * 128:(ti + 1) * 128],
                                 rhs=wpf[:], start=True, stop=True)
            ob = wk.tile([128, 512], f32, tag="ob", bufs=5,
                         name=f"ob{ti}")
            nc.vector.tensor_copy(ob[:], psP[:])
            nc.sync.dma_start(out_d[ti * 128:(ti + 1) * 128, :], ob[:])

        # attention blocks, qi-major; alternate 3-chunk / 2-chunk PSUM slots
        class Blk:
            pass

        blocks = []
        slot = 0
        for qi in range(NQT):
            for h in range(2):
                nkc = 4 * qi + 4
                kc = 0
                while kc < nkc:
                    b = Blk()
                    b.h, b.qi, b.kc = h, qi, kc
                    b.len = min(3, nkc - kc)
                    b.slot = slot
                    b.first = kc == 0
                    b.last = kc + b.len == nkc
                    blocks.append(b)
                    slot ^= 1
                    kc += b.len

        psO = {}

        def emit_s(b):
            hb = b.h * 64
            b.psS = sp.tile([128, 1536], f32, tag=f"s{b.slot}", bufs=1,
                            name=f"s_{b.h}_{b.qi}_{b.kc}")
            for j in range(b.len):
                p = (b.kc + j) - 4 * b.qi
                # diag chunks p=1,2: only the valid q-suffix is ever read
                # downstream (exp/mask/PV all shrunk); p=3 stays full
                # (f32r below N=256 runs at 4 cyc/row - no win).
                off = 128 * p if p in (1, 2) else 0
                nc.tensor.matmul(
                    b.psS[:, j * 512 + off:(j + 1) * 512],
                    lhsT=kT[hb:hb + 64,
                            (b.kc + j) * 128:(b.kc + j + 1) * 128],
                    rhs=qT[hb:hb + 64,
                           b.qi * 512 + off:(b.qi + 1) * 512],
                    start=True, stop=True)

        def emit_f(b):
            h, qi = b.h, b.qi
            nkc = 4 * qi + 4
            va = vaug[h]
            P = pp.tile([128, b.len * 512], pdt, tag="p",
                        name=f"p_{h}_{qi}_{b.kc}")
            p0 = b.kc - 4 * qi
            off0 = 128 * p0 if p0 >= 1 else 0
            nc.scalar.activation(P[:, off0:], b.psS[:, off0:b.len * 512],
                                 Exp)
            for j in range(b.len):
                p = (b.kc + j) - 4 * qi
                if p >= 0:
                    off = 128 * p
                    nc.vector.tensor_mul(
                        P[:, j * 512 + off:(j + 1) * 512],
                        P[:, j * 512 + off:(j + 1) * 512],
                        mask[:, p * 512 + off:(p + 1) * 512])
            if b.first:
                psO[(h, qi)] = po_pool.tile([128, 512], f32, tag="po",
                                            name=f"o_{h}_{qi}")
            po = psO[(h, qi)]
            for j in range(b.len):
                p = (b.kc + j) - 4 * qi
                off = 128 * p if p >= 1 else 0
                nc.tensor.matmul(
                    po[0:65, off:],
                    lhsT=va[:, (b.kc + j) * 65:(b.kc + j) * 65 + 65],
                    rhs=P[:, j * 512 + off:(j + 1) * 512],
                    start=(b.kc + j == 0), stop=(b.kc + j == nkc - 1))
            if b.last:
                # stage psO to SBUF fast so the PSUM slot frees early
                oU = wk.tile([65, 512], f32, tag="oU", bufs=3,
                             name=f"oU{h}_{qi}")
                nc.vector.tensor_copy(oU[:], po[0:65, :])
                rec0 = wk.tile([1, 512], f32, tag="rec0", bufs=3,
                               name=f"rc0{h}_{qi}")
                eng0 = nc.scalar if (h == 1 and qi == NQT - 1) else nc.sync
                eng0.dma_start(rec0[:], oU[64:65, :])
                rec = wk.tile([1, 512], f32, tag="rec", bufs=3,
                               name=f"rc{h}_{qi}")
                nc.vector.reciprocal_approx_fast(rec[:], rec0[:])
                rb = wk.tile([64, 512], f32, tag="rb", bufs=3,
                              name=f"rb{h}_{qi}")
                nc.gpsimd.partition_broadcast(rb[:], rec[:])
                dstq = slice(qi * 512, (qi + 1) * 512)
                if h == 0:
                    nc.vector.tensor_mul(oTS[0:64, dstq], oU[0:64, :],
                                         rb[:])
                else:
                    nc.vector.tensor_mul(oT1[:, dstq], oU[0:64, :], rb[:])
                    last = qi == NQT - 1
                    if not last:
                        nc.sync.dma_start(oTS[64:128, dstq], oT1[:, dstq])
                    for ti in range(4 * qi, 4 * qi + 4):
                        alt = last and ti % 2 == 1
                        pending.append(
                            lambda ti=ti, alt=alt, sp_=last:
                            emit_proj(ti, alt, sp_))

        # main pipelined emission: S of block i+1 before finish of block i;
        # qk/v/proj groups spread between attention blocks via pending queue
        pending = []

        def emit_inputs(t):
            if t >= NQT:
                return
            if t + 1 < NQT:
                sn = slice((t + 1) * 512, (t + 2) * 512)
                for i in range(4):
                    nc.sync.dma_start(xtf[i][:, sn],
                                      xt_d[i * 128:(i + 1) * 128, sn])
            emit_qk(0, t)
            emit_qk(1, t)
            s = slice(t * 512, (t + 1) * 512)
            xit = [wk.tile([128, 512], bf16, tag=f"xit{i}", bufs=2,
                           name=f"xit{i}_{t}") for i in range(4)]
            for i in range(4):
                nc.vector.tensor_copy(xit[i][:], xtf[i][:, s].bitcast(f32))
            for tv in range(4 * t, 4 * t + 4):
                pending.append(lambda tv=tv, xit=xit: emit_v(tv, xit))

        emit_inputs(0)
        for tv in range(0, 4):
            pending.pop(0)()
        bi = 0
        from collections import deque
        prevq = deque()
        LOOKAHEAD = 3
        for t in range(NQT):
            if pending:
                for fn in pending:
                    fn()
                pending.clear()
            emit_inputs(t + 1)
            while bi < len(blocks) and blocks[bi].qi == t:
                b = blocks[bi]
                emit_s(b)
                prevq.append(b)
                if len(prevq) > LOOKAHEAD:
                    emit_f(prevq.popleft())
                bi += 1
                if pending:
                    pending.pop(0)()
        while prevq:
            emit_f(prevq.popleft())
        for fn in pending:
            fn()
        pending.clear()

        if dbg:
            nc.sync.dma_start(dbg_qt[:], qT[:])
            nc.sync.dma_start(dbg_kt[:], kT[:])
            nc.sync.dma_start(dbg_va[:], vaug[1][:])
            nc.sync.dma_start(dbg_ot[:], oTS[:])

    nc.compile()
    return nc


def _get_nc(has_bias=True):
    key = f"nc{has_bias}"
    if key not in _cache:
        _cache[key] = _build(has_bias)
    return _cache[key]


def _prep_inputs(x, w_qkv, b_qkv, w_proj):
    x = np.asarray(x, np.float32)
    w_qkv = np.asarray(w_qkv, np.float32)
    b_qkv = np.asarray(b_qkv, np.float32)
    bf = ml_dtypes.bfloat16
    pdt_np = bf if MODE == "fast" else np.float32

    # causal mask tile: mask[k, p*512 + q] = (128*p + k <= q)
    k_idx = np.arange(128)[:, None]
    q_idx = np.arange(512)[None, :]
    mask = np.concatenate(
        [(128 * p + k_idx <= q_idx) for p in range(4)], axis=1)
    mask = mask.astype(pdt_np)

    in_maps = []
    for c in range(8):
        b = c // 4
        h0 = 2 * (c % 4)
        cols = slice(h0 * 64, (h0 + 2) * 64)  # 128 contiguous dims (2 heads)
        xt = np.ascontiguousarray(x[b].T)
        wq = w_qkv[:, :C][:, cols] * 0.125
        wkk = w_qkv[:, C:2 * C][:, cols]
        wvv = w_qkv[:, 2 * C:][:, cols]
        bq = b_qkv[:C][cols] * 0.125
        bk = b_qkv[C:2 * C][cols]
        bvv = b_qkv[2 * C:][cols]
        in_maps.append({
            "xt": xt,
            "wqk": np.ascontiguousarray(np.concatenate([wq, wkk], axis=1)),
            "wv": np.ascontiguousarray(wvv.astype(bf)),
            "bqk": np.ascontiguousarray(np.stack([bq, bk], axis=1)),
            "bv": np.ascontiguousarray(bvv[None, :].astype(bf)),
            "wp": np.ascontiguousarray(
                np.asarray(w_proj, np.float32)[cols, :]),
            "mask": mask,
        })
    return in_maps


def kernel(x, w_qkv, b_qkv, w_proj, b_proj, _want_trace=False):
    from concourse.bass_utils import run_bass_kernel_spmd

    has_bias = bool(np.any(np.asarray(b_qkv)))
    nc = _get_nc(has_bias)
    in_maps = _prep_inputs(x, w_qkv, b_qkv, w_proj)
    res = run_bass_kernel_spmd(nc, in_maps, list(range(8)),
                               trace=_want_trace)
    if _want_trace:
        _cache["last_result"] = res
    out = np.zeros((B, T, C), np.float32)
    for c in range(8):
        out[c // 4] += res.results[c]["partial"]
    out += np.asarray(b_proj, np.float32)[None, None, :]
    return out


# revision 42
# speedup vs baseline: 1.0193x; 1.0004x over previous
"""Multi-head causal attention (B=2, T=4096, H=8, D=64) on 8 TRN2 NeuronCores.

Sharding: core c handles batch b = c//4 and heads (2*(c%4), 2*(c%4)+1).
Each core computes QKV for its 2 heads, causal flash-attention in an
S^T layout (keys on partitions, queries on free dim; exp on ACT; softmax
denominators via a ones-augmented V matmul), and its partial output
projection. Host sums the 4 per-batch partials and adds b_proj.

The attention inner loop is software-pipelined: S^T matmul blocks are
emitted one block ahead of their exp/mask/PV consumers so the PE stream
does not stall on ACT. PSUM budget (8 banks): S^T slots 3+2 (alternating
3-chunk/2-chunk blocks), psO accumulators 2, shared qkv/v/proj slot 1.
"""

import os
import sys

for _p in ("/opt/trn_rl_repo", "/root/.axon_site/_ro/trn_rl_repo"):
    if os.path.isdir(_p) and _p not in sys.path:
        sys.path.insert(0, _p)
        break

from contextlib import ExitStack

import ml_dtypes
import numpy as np

B, T, H, D = 2, 4096, 8, 64
C = H * D  # 512
NQT = T // 512  # 8 q-tiles of 512 queries
NKC = T // 128  # 32 k-chunks of 128 keys

# "fast": P and V in bf16 (half SBUF, same PE cost).
MODE = os.environ.get("ATTN_MODE", "fast")

_cache = {}


def _build(has_bias=True):
    import concourse.mybir as mybir
    import concourse.tile as tile
    from concourse import bacc

    f32 = mybir.dt.float32
    f32r = mybir.dt.float32r
    bf16 = mybir.dt.bfloat16
    pdt = bf16 if MODE == "fast" else f32
    Exp = mybir.ActivationFunctionType.Exp

    nc = bacc.Bacc("TRN2", target_bir_lowering=False, debug=False,
                   enable_asserts=False)

    xt_d = nc.dram_tensor("xt", [C, T], f32r, kind="ExternalInput").ap()
    wqk_d = nc.dram_tensor("wqk", [C, 256], f32r, kind="ExternalInput").ap()
    wv_d = nc.dram_tensor("wv", [C, 128], bf16, kind="ExternalInput").ap()
    bqk_d = nc.dram_tensor("bqk", [128, 2], f32, kind="ExternalInput").ap()
    bv_d = nc.dram_tensor("bv", [1, 128], bf16, kind="ExternalInput").ap()
    wp_d = nc.dram_tensor("wp", [128, C], f32r, kind="ExternalInput").ap()
    mask_d = nc.dram_tensor("mask", [128, 2048], pdt,
                            kind="ExternalInput").ap()
    out_d = nc.dram_tensor("partial", [T, C], f32, kind="ExternalOutput").ap()
    dbg = os.environ.get("ATTN_DEBUG") == "1"
    if dbg:
        dbg_qt = nc.dram_tensor("dbg_qt", [128, T], f32r,
                                kind="ExternalOutput").ap()
        dbg_kt = nc.dram_tensor("dbg_kt", [128, T], f32r,
                                kind="ExternalOutput").ap()
        dbg_va = nc.dram_tensor("dbg_va", [128, NKC * 65], pdt,
                                kind="ExternalOutput").ap()
        dbg_ot = nc.dram_tensor("dbg_ot", [128, T], f32r,
                                kind="ExternalOutput").ap()

    with tile.TileContext(nc, trace_sim=False) as tc, ExitStack() as ctx:
        cp = ctx.enter_context(tc.tile_pool(name="const", bufs=1))
        acc = ctx.enter_context(tc.tile_pool(name="acc", bufs=1,
                                             space="PSUM"))
        po_pool = ctx.enter_context(tc.tile_pool(name="po", bufs=1,
                                                 space="PSUM"))
        sp = ctx.enter_context(tc.tile_pool(name="spsum", bufs=1,
                                            space="PSUM"))
        pp = ctx.enter_context(tc.tile_pool(name="pbuf", bufs=5))
        wk = ctx.enter_context(tc.tile_pool(name="wrk", bufs=2))

        def const(shape, dt, tag):
            return cp.tile(shape, dt, tag=tag, name=tag)

        xtf = [const([128, T], f32r, f"xtf{i}") for i in range(4)]
        wqk = [const([128, 256], f32r, f"wqk{i}") for i in range(4)]
        wv = [const([128, 128], bf16, f"wv{i}") for i in range(4)]
        bqk = const([128, 2], f32, "bqk")
        bv = const([1, 128], bf16, "bv")
        wpf = const([128, C], f32r, "wpf")
        wp1 = const([64, C], f32r, "wp1")
        mask = const([128, 2048], pdt, "mask")
        ones1 = const([1, 128], bf16, "ones1")
        qT = const([128, T], f32r, "qT")
        kT = const([128, T], f32r, "kT")
        vaug = [const([128, NKC * 65], pdt, f"vaug{h}") for h in range(2)]
        oTS = const([128, T], f32r, "oTS")
        oT1 = const([64, T], f32r, "oT1")

        # DMA order = first-use order: per-c-chunk interleave so the
        # first qk matmul starts after ~384KB instead of ~1.5MB
        s0 = slice(0, 512)
        for i in range(4):
            nc.sync.dma_start(wqk[i][:], wqk_d[i * 128:(i + 1) * 128, :])
            nc.sync.dma_start(xtf[i][:, s0], xt_d[i * 128:(i + 1) * 128, s0])
        nc.sync.dma_start(bqk[:], bqk_d[:])
        for i in range(4):
            nc.sync.dma_start(wv[i][:], wv_d[i * 128:(i + 1) * 128, :])
        nc.sync.dma_start(bv[:], bv_d[:])
        nc.sync.dma_start(mask[:], mask_d[:])
        s1 = slice(512, 1024)
        for i in range(4):
            nc.sync.dma_start(xtf[i][:, s1], xt_d[i * 128:(i + 1) * 128, s1])
        nc.sync.dma_start(wpf[:], wp_d[:])
        nc.sync.dma_start(wp1[:], wp_d[64:128, :])
        nc.vector.memset(ones1[:], 1.0)
        nc.vector.memset(vaug[0][:], 1.0)
        nc.vector.memset(vaug[1][:], 1.0)

        def emit_qk(g, t):
            dst = qT if g == 0 else kT
            ps = acc.tile([128, 512], f32, tag="acc", name=f"qk{g}_{t}")
            for ci in range(4):
                nc.tensor.matmul(
                    ps[:],
                    lhsT=wqk[ci][:, g * 128:(g + 1) * 128],
                    rhs=xtf[ci][:, t * 512:(t + 1) * 512],
                    start=(ci == 0), stop=(ci == 3))
            if has_bias:
                nc.vector.tensor_scalar_add(
                    dst[:, t * 512:(t + 1) * 512], ps[:], bqk[:, g:g + 1])
            else:
                nc.vector.tensor_copy(dst[:, t * 512:(t + 1) * 512], ps[:])

        def emit_v(t, xit):
            ps = acc.tile([128, 512], f32, tag="acc", name=f"v{t}")
            psv = ps[:, 0:128]
            o = (t % 4) * 128
            for ci in range(4):
                nc.tensor.matmul(
                    psv, lhsT=xit[ci][:, o:o + 128],
                    rhs=wv[ci][:], start=(ci == 0),
                    stop=(not has_bias and ci == 3))
            if has_bias:
                nc.tensor.matmul(psv, lhsT=ones1[:], rhs=bv[:],
                                 start=False, stop=True)
            for h in range(2):
                nc.vector.tensor_copy(vaug[h][:, t * 65:t * 65 + 64],
                                      psv[:, h * 64:h * 64 + 64])

        def emit_proj(ti, alt=False, split=False):
            tagn = "po" if alt else "acc"
            pooln = po_pool if alt else acc
            psP = pooln.tile([128, 512], f32, tag=tagn, name=f"pj{ti}")
            if split:
                # final row: read heads separately so the projs don't wait
                # on the oTS stacking DMA (PE is idle in the tail anyway)
                tc0 = slice(ti * 128, (ti + 1) * 128)
                nc.tensor.matmul(psP[:], lhsT=oTS[0:64, tc0],
                                 rhs=wpf[0:64, :], start=True, stop=False)
                nc.tensor.matmul(psP[:], lhsT=oT1[:, tc0],
                                 rhs=wp1[:], start=False, stop=True)
            else:
                nc.tensor.matmul(psP[:],
                                 lhsT=oTS[:, ti * 128:(ti + 1) * 128],
                                 rhs=wpf[:], start=True, stop=True)
            ob = wk.tile([128, 512], f32, tag="ob", bufs=5,
                         name=f"ob{ti}")
            if split and ti % 2 == 1:
                nc.scalar.copy(ob[:], psP[:])
                nc.scalar.dma_start(out_d[ti * 128:(ti + 1) * 128, :],
                                    ob[:])
            else:
                nc.vector.tensor_copy(ob[:], psP[:])
                nc.sync.dma_start(out_d[ti * 128:(ti + 1) * 128, :], ob[:])

        # attention blocks, qi-major; alternate 3-chunk / 2-chunk PSUM slots
        class Blk:
            pass

        blocks = []
        slot = 0
        for qi in range(NQT):
            for h in range(2):
                nkc = 4 * qi + 4
                kc = 0
                while kc < nkc:
                    b = Blk()
                    b.h, b.qi, b.kc = h, qi, kc
                    b.len = min(3, nkc - kc)
                    b.slot = slot
                    b.first = kc == 0
                    b.last = kc + b.len == nkc
                    blocks.append(b)
                    slot ^= 1
                    kc += b.len

        psO = {}

        def emit_s(b):
            hb = b.h * 64
            b.psS = sp.tile([128, 1536], f32, tag=f"s{b.slot}", bufs=1,
                            name=f"s_{b.h}_{b.qi}_{b.kc}")
            for j in range(b.len):
                p = (b.kc + j) - 4 * b.qi
                # diag chunks p=1,2: only the valid q-suffix is ever read
                # downstream (exp/mask/PV all shrunk); p=3 stays full
                # (f32r below N=256 runs at 4 cyc/row - no win).
                off = 128 * p if p in (1, 2) else 0
                nc.tensor.matmul(
                    b.psS[:, j * 512 + off:(j + 1) * 512],
                    lhsT=kT[hb:hb + 64,
                            (b.kc + j) * 128:(b.kc + j + 1) * 128],
                    rhs=qT[hb:hb + 64,
                           b.qi * 512 + off:(b.qi + 1) * 512],
                    start=True, stop=True)

        def emit_f(b):
            h, qi = b.h, b.qi
            nkc = 4 * qi + 4
            va = vaug[h]
            P = pp.tile([128, b.len * 512], pdt, tag="p",
                        name=f"p_{h}_{qi}_{b.kc}")
            p0 = b.kc - 4 * qi
            off0 = 128 * p0 if p0 >= 1 else 0
            nc.scalar.activation(P[:, off0:], b.psS[:, off0:b.len * 512],
                                 Exp)
            for j in range(b.len):
                p = (b.kc + j) - 4 * qi
                if p >= 0:
                    off = 128 * p
                    nc.vector.tensor_mul(
                        P[:, j * 512 + off:(j + 1) * 512],
                        P[:, j * 512 + off:(j + 1) * 512],
                        mask[:, p * 512 + off:(p + 1) * 512])
            if b.first:
                psO[(h, qi)] = po_pool.tile([128, 512], f32, tag="po",
                                            name=f"o_{h}_{qi}")
            po = psO[(h, qi)]
            for j in range(b.len):
                p = (b.kc + j) - 4 * qi
                off = 128 * p if p >= 1 else 0
                nc.tensor.matmul(
                    po[0:65, off:],
                    lhsT=va[:, (b.kc + j) * 65:(b.kc + j) * 65 + 65],
                    rhs=P[:, j * 512 + off:(j + 1) * 512],
                    start=(b.kc + j == 0), stop=(b.kc + j == nkc - 1))
            if b.last:
                # stage psO to SBUF fast so the PSUM slot frees early
                oU = wk.tile([65, 512], f32, tag="oU", bufs=3,
                             name=f"oU{h}_{qi}")
                nc.vector.tensor_copy(oU[:], po[0:65, :])
                rec0 = wk.tile([1, 512], f32, tag="rec0", bufs=3,
                               name=f"rc0{h}_{qi}")
                eng0 = nc.scalar if (h == 1 and qi == NQT - 1) else nc.sync
                eng0.dma_start(rec0[:], oU[64:65, :])
                rec = wk.tile([1, 512], f32, tag="rec", bufs=3,
                               name=f"rc{h}_{qi}")
                nc.vector.reciprocal_approx_fast(rec[:], rec0[:])
                rb = wk.tile([64, 512], f32, tag="rb", bufs=3,
                              name=f"rb{h}_{qi}")
                nc.gpsimd.partition_broadcast(rb[:], rec[:])
                dstq = slice(qi * 512, (qi + 1) * 512)
                if h == 0:
                    nc.vector.tensor_mul(oTS[0:64, dstq], oU[0:64, :],
                                         rb[:])
                else:
                    nc.vector.tensor_mul(oT1[:, dstq], oU[0:64, :], rb[:])
                    last = qi == NQT - 1
                    if not last:
                        nc.sync.dma_start(oTS[64:128, dstq], oT1[:, dstq])
                    for ti in range(4 * qi, 4 * qi + 4):
                        alt = last and ti % 2 == 1
                        pending.append(
                            lambda ti=ti, alt=alt, sp_=last:
                            emit_proj(ti, alt, sp_))

        # main pipelined emission: S of block i+1 before finish of block i;
        # qk/v/proj groups spread between attention blocks via pending queue
        pending = []

        def emit_inputs(t):
            if t >= NQT:
                return
            if t + 1 < NQT:
                sn = slice((t + 1) * 512, (t + 2) * 512)
                for i in range(4):
                    nc.sync.dma_start(xtf[i][:, sn],
                                      xt_d[i * 128:(i + 1) * 128, sn])
            emit_qk(0, t)
            emit_qk(1, t)
            s = slice(t * 512, (t + 1) * 512)
            xit = [wk.tile([128, 512], bf16, tag=f"xit{i}", bufs=2,
                           name=f"xit{i}_{t}") for i in range(4)]
            for i in range(4):
                nc.vector.tensor_copy(xit[i][:], xtf[i][:, s].bitcast(f32))
            for tv in range(4 * t, 4 * t + 4):
                pending.append(lambda tv=tv, xit=xit: emit_v(tv, xit))

        emit_inputs(0)
        for tv in range(0, 4):
            pending.pop(0)()
        bi = 0
        from collections import deque
        prevq = deque()
        LOOKAHEAD = 3
        for t in range(NQT):
            if pending:
                for fn in pending:
                    fn()
                pending.clear()
            emit_inputs(t + 1)
            while bi < len(blocks) and blocks[bi].qi == t:
                b = blocks[bi]
                emit_s(b)
                prevq.append(b)
                if len(prevq) > LOOKAHEAD:
                    emit_f(prevq.popleft())
                bi += 1
                if pending:
                    pending.pop(0)()
        while prevq:
            emit_f(prevq.popleft())
        for fn in pending:
            fn()
        pending.clear()

        if dbg:
            nc.sync.dma_start(dbg_qt[:], qT[:])
            nc.sync.dma_start(dbg_kt[:], kT[:])
            nc.sync.dma_start(dbg_va[:], vaug[1][:])
            nc.sync.dma_start(dbg_ot[:], oTS[:])

    nc.compile()
    return nc


def _get_nc(has_bias=True):
    key = f"nc{has_bias}"
    if key not in _cache:
        _cache[key] = _build(has_bias)
    return _cache[key]


def _prep_inputs(x, w_qkv, b_qkv, w_proj):
    x = np.asarray(x, np.float32)
    w_qkv = np.asarray(w_qkv, np.float32)
    b_qkv = np.asarray(b_qkv, np.float32)
    bf = ml_dtypes.bfloat16
    pdt_np = bf if MODE == "fast" else np.float32

    # causal mask tile: mask[k, p*512 + q] = (128*p + k <= q)
    k_idx = np.arange(128)[:, None]
    q_idx = np.arange(512)[None, :]
    mask = np.concatenate(
        [(128 * p + k_idx <= q_idx) for p in range(4)], axis=1)
    mask = mask.astype(pdt_np)

    in_maps = []
    for c in range(8):
        b = c // 4
        h0 = 2 * (c % 4)
        cols = slice(h0 * 64, (h0 + 2) * 64)  # 128 contiguous dims (2 heads)
        xt = np.ascontiguousarray(x[b].T)
        wq = w_qkv[:, :C][:, cols] * 0.125
        wkk = w_qkv[:, C:2 * C][:, cols]
        wvv = w_qkv[:, 2 * C:][:, cols]
        bq = b_qkv[:C][cols] * 0.125
        bk = b_qkv[C:2 * C][cols]
        bvv = b_qkv[2 * C:][cols]
        in_maps.append({
            "xt": xt,
            "wqk": np.ascontiguousarray(np.concatenate([wq, wkk], axis=1)),
            "wv": np.ascontiguousarray(wvv.astype(bf)),
            "bqk": np.ascontiguousarray(np.stack([bq, bk], axis=1)),
            "bv": np.ascontiguousarray(bvv[None, :].astype(bf)),
            "wp": np.ascontiguousarray(
                np.asarray(w_proj, np.float32)[cols, :]),
            "mask": mask,
        })
    return in_maps


def kernel(x, w_qkv, b_qkv, w_proj, b_proj, _want_trace=False):
    from concourse.bass_utils import run_bass_kernel_spmd

    has_bias = bool(np.any(np.asarray(b_qkv)))
    nc = _get_nc(has_bias)
    in_maps = _prep_inputs(x, w_qkv, b_qkv, w_proj)
    res = run_bass_kernel_spmd(nc, in_maps, list(range(8)),
                               trace=_want_trace)
    if _want_trace:
        _cache["last_result"] = res
    out = np.zeros((B, T, C), np.float32)
    for c in range(8):
        out[c // 4] += res.results[c]["partial"]
    out += np.asarray(b_proj, np.float32)[None, None, :]
    return out


# revision 43
# speedup vs baseline: 1.0203x; 1.0010x over previous
"""Multi-head causal attention (B=2, T=4096, H=8, D=64) on 8 TRN2 NeuronCores.

Sharding: core c handles batch b = c//4 and heads (2*(c%4), 2*(c%4)+1).
Each core computes QKV for its 2 heads, causal flash-attention in an
S^T layout (keys on partitions, queries on free dim; exp on ACT; softmax
denominators via a ones-augmented V matmul), and its partial output
projection. Host sums the 4 per-batch partials and adds b_proj.

The attention inner loop is software-pipelined: S^T matmul blocks are
emitted one block ahead of their exp/mask/PV consumers so the PE stream
does not stall on ACT. PSUM budget (8 banks): S^T slots 3+2 (alternating
3-chunk/2-chunk blocks), psO accumulators 2, shared qkv/v/proj slot 1.
"""

import os
import sys

for _p in ("/opt/trn_rl_repo", "/root/.axon_site/_ro/trn_rl_repo"):
    if os.path.isdir(_p) and _p not in sys.path:
        sys.path.insert(0, _p)
        break

from contextlib import ExitStack

import ml_dtypes
import numpy as np

B, T, H, D = 2, 4096, 8, 64
C = H * D  # 512
NQT = T // 512  # 8 q-tiles of 512 queries
NKC = T // 128  # 32 k-chunks of 128 keys

# "fast": P and V in bf16 (half SBUF, same PE cost).
MODE = os.environ.get("ATTN_MODE", "fast")

_cache = {}


def _build(has_bias=True):
    import concourse.mybir as mybir
    import concourse.tile as tile
    from concourse import bacc

    f32 = mybir.dt.float32
    f32r = mybir.dt.float32r
    bf16 = mybir.dt.bfloat16
    pdt = bf16 if MODE == "fast" else f32
    Exp = mybir.ActivationFunctionType.Exp

    nc = bacc.Bacc("TRN2", target_bir_lowering=False, debug=False,
                   enable_asserts=False)

    xt_d = nc.dram_tensor("xt", [C, T], f32r, kind="ExternalInput").ap()
    wqk_d = nc.dram_tensor("wqk", [C, 256], f32r, kind="ExternalInput").ap()
    wv_d = nc.dram_tensor("wv", [C, 128], bf16, kind="ExternalInput").ap()
    bqk_d = nc.dram_tensor("bqk", [128, 2], f32, kind="ExternalInput").ap()
    bv_d = nc.dram_tensor("bv", [1, 128], bf16, kind="ExternalInput").ap()
    wp_d = nc.dram_tensor("wp", [128, C], f32r, kind="ExternalInput").ap()
    mask_d = nc.dram_tensor("mask", [128, 2048], pdt,
                            kind="ExternalInput").ap()
    out_d = nc.dram_tensor("partial", [T, C], f32, kind="ExternalOutput").ap()
    dbg = os.environ.get("ATTN_DEBUG") == "1"
    if dbg:
        dbg_qt = nc.dram_tensor("dbg_qt", [128, T], f32r,
                                kind="ExternalOutput").ap()
        dbg_kt = nc.dram_tensor("dbg_kt", [128, T], f32r,
                                kind="ExternalOutput").ap()
        dbg_va = nc.dram_tensor("dbg_va", [128, NKC * 65], pdt,
                                kind="ExternalOutput").ap()
        dbg_ot = nc.dram_tensor("dbg_ot", [128, T], f32r,
                                kind="ExternalOutput").ap()

    with tile.TileContext(nc, trace_sim=False) as tc, ExitStack() as ctx:
        cp = ctx.enter_context(tc.tile_pool(name="const", bufs=1))
        acc = ctx.enter_context(tc.tile_pool(name="acc", bufs=1,
                                             space="PSUM"))
        po_pool = ctx.enter_context(tc.tile_pool(name="po", bufs=1,
                                                 space="PSUM"))
        sp = ctx.enter_context(tc.tile_pool(name="spsum", bufs=1,
                                            space="PSUM"))
        pp = ctx.enter_context(tc.tile_pool(name="pbuf", bufs=5))
        wk = ctx.enter_context(tc.tile_pool(name="wrk", bufs=2))

        def const(shape, dt, tag):
            return cp.tile(shape, dt, tag=tag, name=tag)

        xtf = [const([128, T], f32r, f"xtf{i}") for i in range(4)]
        wqk = [const([128, 256], f32r, f"wqk{i}") for i in range(4)]
        wv = [const([128, 128], bf16, f"wv{i}") for i in range(4)]
        bqk = const([128, 2], f32, "bqk")
        bv = const([1, 128], bf16, "bv")
        wpf = const([128, C], f32r, "wpf")
        wp1 = const([64, C], f32r, "wp1")
        mask = const([128, 2048], pdt, "mask")
        ones1 = const([1, 128], bf16, "ones1")
        qT = const([128, T], f32r, "qT")
        kT = const([128, T], f32r, "kT")
        vaug = [const([128, NKC * 65], pdt, f"vaug{h}") for h in range(2)]
        oTS = const([128, T], f32r, "oTS")
        oT1 = const([64, T], f32r, "oT1")

        # DMA order = first-use order: per-c-chunk interleave so the
        # first qk matmul starts after ~384KB instead of ~1.5MB
        s0 = slice(0, 512)
        for i in range(4):
            nc.sync.dma_start(wqk[i][:], wqk_d[i * 128:(i + 1) * 128, :])
            nc.sync.dma_start(xtf[i][:, s0], xt_d[i * 128:(i + 1) * 128, s0])
        nc.sync.dma_start(bqk[:], bqk_d[:])
        for i in range(4):
            nc.sync.dma_start(wv[i][:], wv_d[i * 128:(i + 1) * 128, :])
        nc.sync.dma_start(bv[:], bv_d[:])
        nc.sync.dma_start(mask[:], mask_d[:])
        s1 = slice(512, 1024)
        for i in range(4):
            nc.sync.dma_start(xtf[i][:, s1], xt_d[i * 128:(i + 1) * 128, s1])
        nc.sync.dma_start(wpf[:], wp_d[:])
        nc.sync.dma_start(wp1[:], wp_d[64:128, :])
        nc.vector.memset(ones1[:], 1.0)
        nc.vector.memset(vaug[0][:], 1.0)
        nc.vector.memset(vaug[1][:], 1.0)

        def emit_qk(g, t):
            dst = qT if g == 0 else kT
            ps = acc.tile([128, 512], f32, tag="acc", name=f"qk{g}_{t}")
            for ci in range(4):
                nc.tensor.matmul(
                    ps[:],
                    lhsT=wqk[ci][:, g * 128:(g + 1) * 128],
                    rhs=xtf[ci][:, t * 512:(t + 1) * 512],
                    start=(ci == 0), stop=(ci == 3))
            if has_bias:
                nc.vector.tensor_scalar_add(
                    dst[:, t * 512:(t + 1) * 512], ps[:], bqk[:, g:g + 1])
            else:
                nc.vector.tensor_copy(dst[:, t * 512:(t + 1) * 512], ps[:])

        def emit_v(t, xit):
            ps = acc.tile([128, 512], f32, tag="acc", name=f"v{t}")
            psv = ps[:, 0:128]
            o = (t % 4) * 128
            for ci in range(4):
                nc.tensor.matmul(
                    psv, lhsT=xit[ci][:, o:o + 128],
                    rhs=wv[ci][:], start=(ci == 0),
                    stop=(not has_bias and ci == 3))
            if has_bias:
                nc.tensor.matmul(psv, lhsT=ones1[:], rhs=bv[:],
                                 start=False, stop=True)
            for h in range(2):
                nc.vector.tensor_copy(vaug[h][:, t * 65:t * 65 + 64],
                                      psv[:, h * 64:h * 64 + 64])

        def emit_proj(ti, alt=False, split=False):
            tagn = "po" if alt else "acc"
            pooln = po_pool if alt else acc
            psP = pooln.tile([128, 512], f32, tag=tagn, name=f"pj{ti}")
            if split:
                # final row: read heads separately so the projs don't wait
                # on the oTS stacking DMA (PE is idle in the tail anyway)
                tc0 = slice(ti * 128, (ti + 1) * 128)
                nc.tensor.matmul(psP[:], lhsT=oTS[0:64, tc0],
                                 rhs=wpf[0:64, :], start=True, stop=False)
                nc.tensor.matmul(psP[:], lhsT=oT1[:, tc0],
                                 rhs=wp1[:], start=False, stop=True)
            else:
                nc.tensor.matmul(psP[:],
                                 lhsT=oTS[:, ti * 128:(ti + 1) * 128],
                                 rhs=wpf[:], start=True, stop=True)
            ob = wk.tile([128, 512], f32, tag="ob", bufs=5,
                         name=f"ob{ti}")
            if split and ti % 2 == 1:
                nc.scalar.copy(ob[:], psP[:])
                nc.scalar.dma_start(out_d[ti * 128:(ti + 1) * 128, :],
                                    ob[:])
            else:
                nc.vector.tensor_copy(ob[:], psP[:])
                nc.sync.dma_start(out_d[ti * 128:(ti + 1) * 128, :], ob[:])

        # attention blocks, qi-major; alternate 3-chunk / 2-chunk PSUM slots
        class Blk:
            pass

        blocks = []
        slot = 0
        for qi in range(NQT):
            for h in range(2):
                nkc = 4 * qi + 4
                kc = 0
                while kc < nkc:
                    b = Blk()
                    b.h, b.qi, b.kc = h, qi, kc
                    b.len = min(3, nkc - kc)
                    b.slot = slot
                    b.first = kc == 0
                    b.last = kc + b.len == nkc
                    blocks.append(b)
                    slot ^= 1
                    kc += b.len

        psO = {}

        def emit_s(b):
            hb = b.h * 64
            b.psS = sp.tile([128, 1536], f32, tag=f"s{b.slot}", bufs=1,
                            name=f"s_{b.h}_{b.qi}_{b.kc}")
            for j in range(b.len):
                p = (b.kc + j) - 4 * b.qi
                # diag chunks p=1,2: only the valid q-suffix is ever read
                # downstream (exp/mask/PV all shrunk); p=3 stays full
                # (f32r below N=256 runs at 4 cyc/row - no win).
                off = 128 * p if p in (1, 2) else 0
                nc.tensor.matmul(
                    b.psS[:, j * 512 + off:(j + 1) * 512],
                    lhsT=kT[hb:hb + 64,
                            (b.kc + j) * 128:(b.kc + j + 1) * 128],
                    rhs=qT[hb:hb + 64,
                           b.qi * 512 + off:(b.qi + 1) * 512],
                    start=True, stop=True)

        def emit_f(b):
            h, qi = b.h, b.qi
            nkc = 4 * qi + 4
            va = vaug[h]
            P = pp.tile([128, b.len * 512], pdt, tag="p",
                        name=f"p_{h}_{qi}_{b.kc}")
            p0 = b.kc - 4 * qi
            off0 = 128 * p0 if p0 >= 1 else 0
            nc.scalar.activation(P[:, off0:], b.psS[:, off0:b.len * 512],
                                 Exp)
            for j in range(b.len):
                p = (b.kc + j) - 4 * qi
                if p >= 0:
                    off = 128 * p
                    nc.vector.tensor_mul(
                        P[:, j * 512 + off:(j + 1) * 512],
                        P[:, j * 512 + off:(j + 1) * 512],
                        mask[:, p * 512 + off:(p + 1) * 512])
            if b.first:
                psO[(h, qi)] = po_pool.tile([128, 512], f32, tag="po",
                                            name=f"o_{h}_{qi}")
            po = psO[(h, qi)]
            for j in range(b.len):
                p = (b.kc + j) - 4 * qi
                off = 128 * p if p >= 1 else 0
                nc.tensor.matmul(
                    po[0:65, off:],
                    lhsT=va[:, (b.kc + j) * 65:(b.kc + j) * 65 + 65],
                    rhs=P[:, j * 512 + off:(j + 1) * 512],
                    start=(b.kc + j == 0), stop=(b.kc + j == nkc - 1))
            if b.last:
                # stage psO to SBUF fast so the PSUM slot frees early
                oU = wk.tile([65, 512], f32, tag="oU", bufs=3,
                             name=f"oU{h}_{qi}")
                nc.vector.tensor_copy(oU[:], po[0:65, :])
                rec0 = wk.tile([1, 512], f32, tag="rec0", bufs=3,
                               name=f"rc0{h}_{qi}")
                eng0 = nc.scalar if (h == 1 and qi == NQT - 1) else nc.sync
                eng0.dma_start(rec0[:], oU[64:65, :])
                rec = wk.tile([1, 512], f32, tag="rec", bufs=3,
                               name=f"rc{h}_{qi}")
                nc.vector.reciprocal_approx_fast(rec[:], rec0[:])
                rb = wk.tile([64, 512], f32, tag="rb", bufs=3,
                              name=f"rb{h}_{qi}")
                nc.gpsimd.partition_broadcast(rb[:], rec[:])
                dstq = slice(qi * 512, (qi + 1) * 512)
                if h == 0:
                    nc.vector.tensor_mul(oTS[0:64, dstq], oU[0:64, :],
                                         rb[:])
                else:
                    nc.vector.tensor_mul(oT1[:, dstq], oU[0:64, :], rb[:])
                    last = qi == NQT - 1
                    if not last:
                        nc.sync.dma_start(oTS[64:128, dstq], oT1[:, dstq])
                    for ti in range(4 * qi, 4 * qi + 4):
                        alt = last and ti % 2 == 1
                        pending.append(
                            lambda ti=ti, alt=alt, sp_=last:
                            emit_proj(ti, alt, sp_))

        # main pipelined emission: S of block i+1 before finish of block i;
        # qk/v/proj groups spread between attention blocks via pending queue
        pending = []

        def emit_inputs(t, defer=False):
            if t >= NQT:
                return
            if t + 1 < NQT:
                # DMA issues stay inline (queue early, transfer async)
                sn = slice((t + 1) * 512, (t + 2) * 512)
                for i in range(4):
                    nc.sync.dma_start(xtf[i][:, sn],
                                      xt_d[i * 128:(i + 1) * 128, sn])
            s = slice(t * 512, (t + 1) * 512)
            xit = [wk.tile([128, 512], bf16, tag=f"xit{i}", bufs=2,
                           name=f"xit{i}_{t}") for i in range(4)]

            def qk_and_casts(g):
                emit_qk(g, t)
                for i in (g * 2, g * 2 + 1):
                    nc.vector.tensor_copy(xit[i][:],
                                          xtf[i][:, s].bitcast(f32))

            if defer:
                pending.append(lambda: qk_and_casts(0))
                pending.append(lambda: qk_and_casts(1))
            else:
                qk_and_casts(0)
                qk_and_casts(1)
            for tv in range(4 * t, 4 * t + 4):
                pending.append(lambda tv=tv, xit=xit: emit_v(tv, xit))

        emit_inputs(0)
        for tv in range(0, 4):
            pending.pop(0)()
        bi = 0
        from collections import deque
        prevq = deque()
        LOOKAHEAD = 3
        for t in range(NQT):
            if pending:
                for fn in pending:
                    fn()
                pending.clear()
            emit_inputs(t + 1, defer=True)
            while bi < len(blocks) and blocks[bi].qi == t:
                b = blocks[bi]
                emit_s(b)
                prevq.append(b)
                if len(prevq) > LOOKAHEAD:
                    emit_f(prevq.popleft())
                bi += 1
                if pending:
                    pending.pop(0)()
        while prevq:
            emit_f(prevq.popleft())
        for fn in pending:
            fn()
        pending.clear()

        if dbg:
            nc.sync.dma_start(dbg_qt[:], qT[:])
            nc.sync.dma_start(dbg_kt[:], kT[:])
            nc.sync.dma_start(dbg_va[:], vaug[1][:])
            nc.sync.dma_start(dbg_ot[:], oTS[:])

    nc.compile()
    return nc


def _get_nc(has_bias=True):
    key = f"nc{has_bias}"
    if key not in _cache:
        _cache[key] = _build(has_bias)
    return _cache[key]


def _prep_inputs(x, w_qkv, b_qkv, w_proj):
    x = np.asarray(x, np.float32)
    w_qkv = np.asarray(w_qkv, np.float32)
    b_qkv = np.asarray(b_qkv, np.float32)
    bf = ml_dtypes.bfloat16
    pdt_np = bf if MODE == "fast" else np.float32

    # causal mask tile: mask[k, p*512 + q] = (128*p + k <= q)
    k_idx = np.arange(128)[:, None]
    q_idx = np.arange(512)[None, :]
    mask = np.concatenate(
        [(128 * p + k_idx <= q_idx) for p in range(4)], axis=1)
    mask = mask.astype(pdt_np)

    in_maps = []
    for c in range(8):
        b = c // 4
        h0 = 2 * (c % 4)
        cols = slice(h0 * 64, (h0 + 2) * 64)  # 128 contiguous dims (2 heads)
        xt = np.ascontiguousarray(x[b].T)
        wq = w_qkv[:, :C][:, cols] * 0.125
        wkk = w_qkv[:, C:2 * C][:, cols]
        wvv = w_qkv[:, 2 * C:][:, cols]
        bq = b_qkv[:C][cols] * 0.125
        bk = b_qkv[C:2 * C][cols]
        bvv = b_qkv[2 * C:][cols]
        in_maps.append({
            "xt": xt,
            "wqk": np.ascontiguousarray(np.concatenate([wq, wkk], axis=1)),
            "wv": np.ascontiguousarray(wvv.astype(bf)),
            "bqk": np.ascontiguousarray(np.stack([bq, bk], axis=1)),
            "bv": np.ascontiguousarray(bvv[None, :].astype(bf)),
            "wp": np.ascontiguousarray(
                np.asarray(w_proj, np.float32)[cols, :]),
            "mask": mask,
        })
    return in_maps


def kernel(x, w_qkv, b_qkv, w_proj, b_proj, _want_trace=False):
    from concourse.bass_utils import run_bass_kernel_spmd

    has_bias = bool(np.any(np.asarray(b_qkv)))
    nc = _get_nc(has_bias)
    in_maps = _prep_inputs(x, w_qkv, b_qkv, w_proj)
    res = run_bass_kernel_spmd(nc, in_maps, list(range(8)),
                               trace=_want_trace)
    if _want_trace:
        _cache["last_result"] = res
    out = np.zeros((B, T, C), np.float32)
    for c in range(8):
        out[c // 4] += res.results[c]["partial"]
    out += np.asarray(b_proj, np.float32)[None, None, :]
    return out
